# revision 1
# baseline (speedup 1.0000x reference)
"""Trainium2 Bass kernel: GQA causal self-attention with ALiBi.

Problem: B=4, T=2048, C=2048, 16 Q heads / 4 KV heads, head_dim=128, fp32.

Sharding (8 cores): DP2 x TP4. Core c = (bg, g) with bg = c//4 (batches
2bg, 2bg+1), g = c%4 (KV group g = Q heads 4g..4g+3 + KV head g). The
reference's ALiBi slope is constant within a KV group (slopes[h//4]), so
each core has a single slope. Host feeds x^T per batch (transpose-free
dataflow on chip) and sums the 4 partial Wo outputs per batch.

Numerics: logits are bounded above (~+6) so softmax runs without the
running-max pass. ALiBi decay truncates attention to a 1-prior-key-chunk
window (dropped keys have relative weight < e^-24).

v4 keeps the tensor engine dense (the HAM clock gate re-throttles after
~3.4us of PE idle, halving the PE clock, so stalls cost double):
- alibi+mask fully on PE/ACT: scores arrive in 1/sigma units (Wq
  pre-scaled on host); the per-key term sigma*k rides the ACT exp's
  per-partition bias AP with scale=sigma (both fp32-exact); the
  per-query term + causal mask enter as a second accumulating matmul
  with small-integer bf16 constants (threshold matrix L builds
  -MBIG*relu(k-q+off) ramps + a rank-1 alibi row).
- softmax denom: DVE bf16 adds of p tiles -> all-ones matmul (den
  replicated on all 128 partitions) -> DVE reciprocal_approx_fast
  (single custom op, no iterative divide, no ACT table switches) ->
  DVE multiply. ACT only ever runs Exp/Copy: one table load total.
- chunk m only computes live columns (q >= off): scores/F/exp/acc/PV
  all narrowed; the first (full-width) chunk initializes has_written
  so narrower accumulating matmuls land on clean state.
- software pipelining: engines execute their queues IN ORDER, so the
  output projection of block t is interleaved per-chunk into block
  t+1's attention; the 4 Wo matmuls of one output chunk fill the PE
  while ACT runs exp, and no engine ever waits on the softmax tail.
"""

import math
from contextlib import ExitStack

import ml_dtypes
import numpy as np

import concourse.bass as bass
import concourse.mybir as mybir
import concourse.tile as tile
from concourse import bacc
from concourse.bass_utils import run_bass_kernel_spmd

B, T, C = 4, 2048, 2048
HD = 128          # head dim
HPC = 4           # Q heads per core
QB = 512          # query block (attention tile free dim)
KC = 128          # key chunk
NQB = T // QB     # 4
NCC = C // 128    # 16 contraction chunks for projections
MBIG = 128.0      # mask ramp step in 1/sigma logit units (bf16-exact)
KILL = 120000.0   # column kill (1/sigma units) for q entirely before chunk

F32 = mybir.dt.float32
BF16 = mybir.dt.bfloat16
EXP = mybir.ActivationFunctionType.Exp

_CACHE = {}


def chunks_for(qb):
    """Causal+window key chunks for query block qb (window = 1 prior chunk)."""
    return list(range(max(0, 4 * qb - 1), 4 * qb + 4))


def build_kernel():
    nc = bacc.Bacc(
        "TRN2",
        target_bir_lowering=False,
        debug=False,
        enable_asserts=False,
        num_devices=8,
    )
    xT2 = nc.dram_tensor("xT2", [2, C, T], BF16, kind="ExternalInput").ap()
    wq_d = nc.dram_tensor("wq", [C, HPC * HD], BF16, kind="ExternalInput").ap()
    wk_d = nc.dram_tensor("wk", [C, HD], BF16, kind="ExternalInput").ap()
    wv_d = nc.dram_tensor("wv", [C, HD], BF16, kind="ExternalInput").ap()
    wo_d = nc.dram_tensor("wo", [HPC * HD, C], BF16, kind="ExternalInput").ap()
    L_d = nc.dram_tensor("Lmat", [KC, KC], BF16, kind="ExternalInput").ap()
    U_d = nc.dram_tensor("Umat", [5, KC, QB], BF16, kind="ExternalInput").ap()
    bias_d = nc.dram_tensor("biask", [KC, 1], F32, kind="ExternalInput").ap()
    sig_d = nc.dram_tensor("sigmas", [KC, 1], F32, kind="ExternalInput").ap()
    id_d = nc.dram_tensor("ident", [128, 128], BF16, kind="ExternalInput").ap()
    on_d = nc.dram_tensor("onesc", [128, 128], BF16, kind="ExternalInput").ap()
    outT = nc.dram_tensor("outT", [2, C, T], BF16, kind="ExternalOutput").ap()

    with ExitStack() as ctx:
        tc = ctx.enter_context(tile.TileContext(nc))
        ctx.enter_context(
            nc.allow_low_precision(reason="bf16 inputs, fp32 accumulate")
        )

        consts = ctx.enter_context(tc.tile_pool(name="consts", bufs=1))
        xpool = ctx.enter_context(tc.tile_pool(name="xpool", bufs=34))
        kvpool = ctx.enter_context(tc.tile_pool(name="kvpool", bufs=1))
        qpool = ctx.enter_context(tc.tile_pool(name="qpool", bufs=3))
        ypool = ctx.enter_context(tc.tile_pool(name="ypool", bufs=2))
        apool = ctx.enter_context(tc.tile_pool(name="apool", bufs=4))
        stpool = ctx.enter_context(tc.tile_pool(name="stpool", bufs=4))
        ppool = ctx.enter_context(tc.tile_pool(name="ppool", bufs=6))
        opool = ctx.enter_context(tc.tile_pool(name="opool", bufs=6))
        rpool = ctx.enter_context(tc.tile_pool(name="rpool", bufs=3))

        ps_acc = ctx.enter_context(tc.tile_pool(name="ps_acc", bufs=2, space="PSUM"))
        ps_s = ctx.enter_context(tc.tile_pool(name="ps_s", bufs=2, space="PSUM"))
        ps_y = ctx.enter_context(tc.tile_pool(name="ps_y", bufs=2, space="PSUM"))
        ps_d = ctx.enter_context(tc.tile_pool(name="ps_d", bufs=2, space="PSUM"))

        # block 0's x strip first: the first Q-projection matmul gates on
        # wq + xt[0], so everything the first block doesn't immediately
        # need queues behind these on the DMA queue
        xts0 = []
        for cc in range(NCC):
            xt = xpool.tile([128, QB], BF16, tag="x")
            nc.sync.dma_start(xt, xT2[0, cc * 128:(cc + 1) * 128, 0:QB])
            xts0.append(xt)

        # resident constants / weights
        wq_sb = consts.tile([128, NCC, HPC * HD], BF16)
        nc.sync.dma_start(wq_sb, wq_d.rearrange("(cc p) d -> p cc d", p=128))
        wk_sb = consts.tile([128, NCC, HD], BF16)
        nc.sync.dma_start(wk_sb, wk_d.rearrange("(cc p) d -> p cc d", p=128))
        wv_sb = consts.tile([128, NCC, HD], BF16)
        nc.sync.dma_start(wv_sb, wv_d.rearrange("(cc p) d -> p cc d", p=128))
        wo_sb = consts.tile([128, HPC, C], BF16)
        nc.sync.dma_start(wo_sb, wo_d.rearrange("(hc p) c -> p hc c", p=128))
        bias_sb = consts.tile([128, 1], F32)
        nc.sync.dma_start(bias_sb, bias_d)
        sig_sb = consts.tile([128, 1], F32)
        nc.sync.dma_start(sig_sb, sig_d)
        ident = consts.tile([128, 128], BF16)
        nc.sync.dma_start(ident, id_d)
        ones = consts.tile([128, 128], BF16)
        nc.sync.dma_start(ones, on_d)
        L_sb = consts.tile([128, KC], BF16)
        nc.sync.dma_start(L_sb, L_d)
        U_sb = consts.tile([128, 5, QB], BF16)
        nc.sync.dma_start(U_sb, U_d.rearrange("m p f -> p m f"))

        # K^T / V ring buffers: attention only needs a 5-chunk causal window
        # (1 prior + 4 in-block); 6 slots give one chunk of WAR slack.
        RING = 6
        kt_ring = kvpool.tile([128, RING, KC], BF16, tag="kt")
        v_ring = kvpool.tile([128, RING, HD], BF16, tag="v")

        def emit_oproj_group(prev, co):
            """One output chunk (4 Wo matmuls + evac + store) of the
            deferred output projection for `prev` = (b, t0, y_sb)."""
            pb, pt0, py = prev
            o_ps = ps_acc.tile([128, QB], F32, tag="acc")
            for hc in range(HPC):
                nc.tensor.matmul(
                    o_ps,
                    lhsT=wo_sb[:, hc, co * 128:(co + 1) * 128],
                    rhs=py[:, hc, :],
                    start=(hc == 0),
                    stop=(hc == HPC - 1),
                )
            o_sb = opool.tile([128, QB], BF16, tag="o")
            if co % 2 == 0:
                nc.scalar.copy(o_sb, o_ps)
            else:
                nc.vector.tensor_copy(o_sb, o_ps)
            nc.sync.dma_start(
                outT[pb, co * 128:(co + 1) * 128, pt0:pt0 + QB], o_sb
            )

        prev = None  # (b, t0, y_sb) awaiting output projection
        for b in range(2):
            for tb in range(NQB):
                t0 = tb * QB
                # ---- load x^T strip for this block ----
                if b == 0 and tb == 0:
                    xts = xts0
                else:
                    xts = []
                    for cc in range(NCC):
                        xt = xpool.tile([128, QB], BF16, tag="x")
                        nc.sync.dma_start(
                            xt, xT2[b, cc * 128:(cc + 1) * 128, t0:t0 + QB]
                        )
                        xts.append(xt)

                # ---- projections for this block ----
                qT_sb = qpool.tile([128, HPC, QB], BF16)
                for h in range(HPC):
                    ps = ps_acc.tile([128, QB], F32, tag="acc")
                    for cc in range(NCC):
                        nc.tensor.matmul(
                            ps,
                            lhsT=wq_sb[:, cc, h * HD:(h + 1) * HD],
                            rhs=xts[cc],
                            start=(cc == 0),
                            stop=(cc == NCC - 1),
                        )
                    nc.vector.tensor_copy(qT_sb[:, h, :], ps)

                ps = ps_acc.tile([128, QB], F32, tag="acc")
                for cc in range(NCC):
                    nc.tensor.matmul(
                        ps, lhsT=wk_sb[:, cc, :], rhs=xts[cc],
                        start=(cc == 0), stop=(cc == NCC - 1),
                    )
                for kc in range(4):
                    nc.scalar.copy(
                        kt_ring[:, (tb * 4 + kc) % RING, :],
                        ps[:, kc * KC:(kc + 1) * KC],
                    )

                ps = ps_acc.tile([128, QB], F32, tag="acc")
                for cc in range(NCC):
                    nc.tensor.matmul(
                        ps, lhsT=wv_sb[:, cc, :], rhs=xts[cc],
                        start=(cc == 0), stop=(cc == NCC - 1),
                    )
                vT_tmp = stpool.tile([128, QB], BF16, tag="vt")
                nc.scalar.copy(vT_tmp, ps)
                for kc in range(4):
                    tp = ps_s.tile([128, KC], BF16, tag="s")
                    nc.tensor.transpose(tp, vT_tmp[:, kc * KC:(kc + 1) * KC], ident)
                    nc.vector.tensor_copy(v_ring[:, (tb * 4 + kc) % RING, :], tp)

                # ---- attention for query block qb = tb, with the previous
                # block's output projection interleaved chunk-by-chunk so
                # the PE stays busy while ACT runs exp ----
                qb = tb
                kbs = chunks_for(qb)
                nch = HPC * len(kbs)
                y_sb = ypool.tile([128, HPC, QB], BF16)
                ci = 0
                for h in range(HPC):
                    y_ps = ps_y.tile([128, QB], F32, tag="y")
                    acc = apool.tile([128, QB], BF16, tag="a")
                    for i, kb in enumerate(kbs):
                        m = kb - 4 * qb + 1
                        off = max(0, (m - 1) * KC)  # first live query column
                        s_ps = ps_s.tile([128, QB], F32, tag="s")
                        # scores; first chunk is full width and initializes
                        # the whole bank's has_written bits
                        nc.tensor.matmul(
                            s_ps[:, off:],
                            lhsT=kt_ring[:, kb % RING, :],
                            rhs=qT_sb[:, h, off:],
                            start=True,
                            stop=False,
                        )
                        # alibi (rank-1 per-q row) + causal mask ramp
                        nc.tensor.matmul(
                            s_ps[:, off:],
                            lhsT=L_sb,
                            rhs=U_sb[:, m, off:],
                            start=False,
                            stop=True,
                        )
                        # fill PE while ACT computes exp of this chunk
                        if prev is not None and ci < 16:
                            emit_oproj_group(prev, ci)
                        ci += 1
                        pT = ppool.tile([128, QB], BF16, tag="p")
                        nc.scalar.activation(
                            pT[:, off:], s_ps[:, off:], EXP,
                            bias=bias_sb[:, 0:1], scale=sig_sb[:, 0:1],
                        )
                        if i == 0:
                            nc.vector.tensor_copy(acc, pT)
                        else:
                            nc.vector.tensor_add(
                                acc[:, off:], acc[:, off:], pT[:, off:]
                            )
                        nc.tensor.matmul(
                            y_ps[:, off:],
                            lhsT=v_ring[:, kb % RING, :],
                            rhs=pT[:, off:],
                            start=(i == 0),
                            stop=(i == len(kbs) - 1),
                        )
                    # normalization: den replicated over partitions via
                    # all-ones matmul, fast Newton reciprocal on DVE
                    # (no ACT, no table switches), multiply on DVE.
                    den_ps = ps_d.tile([128, QB], F32, tag="d")
                    nc.tensor.matmul(den_ps, lhsT=ones, rhs=acc)
                    rec = rpool.tile([128, QB], F32, tag="rec")
                    nc.vector.reciprocal_approx_fast(rec, den_ps)
                    nc.vector.tensor_mul(y_sb[:, h, :], y_ps, rec)

                prev = (b, t0, y_sb)

        # drain: output projection of the final block
        for co in range(16):
            emit_oproj_group(prev, co)

    nc.compile()
    return nc


def make_mask_consts():
    """L [128,128] and U [5,128,512] bf16 constants for the mask/alibi
    matmul, in 1/sigma logit units (scores arrive pre-scaled by 1/sigma
    via Wq; the ACT exp applies scale=sigma in fp32):
    (L.T @ U_m)[k, q] = -MBIG*max(0, k - (q - off_m)) [causal ramp]
    + (off_m - q) - KILL*[q < off_m], with off_m = (m-1)*128.
    Row j<127 of L/U is the threshold pair [k >= j+1]*[j >= q-off];
    row 127 carries the rank-1 alibi + column-kill term. All values are
    small integers (bf16-exact wherever the attention weight is
    non-negligible; |off-q|>256 only happens >=129 keys away where
    ALiBi has already zeroed the weight). m=0 is the unmasked prior
    chunk (ramp rows zero)."""
    BF = ml_dtypes.bfloat16
    j = np.arange(KC)[:, None]
    k = np.arange(KC)[None, :]
    q = np.arange(QB)[None, :]
    L = np.zeros((KC, KC), np.float32)
    L[:127] = -MBIG * (k >= (j[:127] + 1))
    L[127, :] = 1.0
    U = np.zeros((5, KC, QB), np.float32)
    for m in range(5):
        off = (m - 1) * KC
        if m >= 1:
            U[m, :127] = (j[:127] >= (q - off))
        U[m, 127, :] = (off - q) - KILL * (q < off)
    return L.astype(BF), U.astype(BF)


def kernel(x, Wq, Wk, Wv, Wo):
    import os
    import time

    dbg = os.environ.get("KERNEL_DEBUG") == "1"
    t0 = time.time()

    def tick(msg):
        nonlocal t0
        if dbg:
            print(f"[kernel] {msg}: {time.time() - t0:.2f}s", flush=True)
        t0 = time.time()

    x = np.ascontiguousarray(np.asarray(x, np.float32))
    Wq = np.ascontiguousarray(np.asarray(Wq, np.float32))
    Wk = np.ascontiguousarray(np.asarray(Wk, np.float32))
    Wv = np.ascontiguousarray(np.asarray(Wv, np.float32))
    Wo = np.ascontiguousarray(np.asarray(Wo, np.float32))

    tick("input prep")
    if "nc" not in _CACHE:
        _CACHE["nc"] = build_kernel()
        tick("build_kernel")
    nc = _CACHE["nc"]

    s = 1.0 / math.sqrt(HD)
    slopes = [2.0 ** -0.5, 0.5, 2.0 ** -1.5, 0.25]
    BF = ml_dtypes.bfloat16
    ident = np.eye(128, dtype=BF)

    L, U = make_mask_consts()
    in_maps = []
    for c in range(8):
        bg, g = c // 4, c % 4
        xT2 = np.stack(
            [np.ascontiguousarray(x[2 * bg + i].T) for i in range(2)]
        )
        biask = (slopes[g] * np.arange(KC, dtype=np.float32))[:, None]
        in_maps.append({
            "xT2": xT2.astype(BF),
            "wq": (Wq[:, g * 512:(g + 1) * 512] * (s / slopes[g])).astype(BF),
            "wk": Wk[:, g * HD:(g + 1) * HD].astype(BF),
            "wv": Wv[:, g * HD:(g + 1) * HD].astype(BF),
            "wo": Wo[g * 512:(g + 1) * 512, :].astype(BF),
            "Lmat": L,
            "Umat": U,
            "biask": biask,
            "sigmas": np.full((KC, 1), slopes[g], np.float32),
            "ident": ident,
            "onesc": np.ones((128, 128), BF),
        })

    tick("in_maps prep")
    res = run_bass_kernel_spmd(nc, in_maps, core_ids=list(range(8)))
    tick("device run")
    out = np.zeros((B, T, C), np.float32)
    for c in range(8):
        bg, g = c // 4, c % 4
        oT = np.asarray(res.results[c]["outT"], np.float32)
        for i in range(2):
            out[2 * bg + i] += oT[i].T
    tick("gather")
    return out



# revision 4
# speedup vs baseline: 1.0759x; 1.0759x over previous
"""Trainium2 Bass kernel: GQA causal self-attention with ALiBi.

Problem: B=4, T=2048, C=2048, 16 Q heads / 4 KV heads, head_dim=128, fp32.

Sharding (8 cores): DP2 x TP4. Core c = (bg, g) with bg = c//4 (batches
2bg, 2bg+1), g = c%4 (KV group g = Q heads 4g..4g+3 + KV head g). The
reference's ALiBi slope is constant within a KV group (slopes[h//4]), so
each core has a single slope. Host feeds x^T per batch (transpose-free
dataflow on chip) and sums the 4 partial Wo outputs per batch.

Numerics: logits are bounded above (~+6) so softmax runs without the
running-max pass. ALiBi decay truncates attention: key chunk kc only
matters for query chunk kc and kc+1 (distance >= 129 keys has relative
weight < e^-32 even at the smallest slope 0.25), so each 128-query
column attends to exactly 2 key chunks (prior + diagonal); truncation
error ~1e-6.

v5 keeps the tensor engine dense (the HAM clock gate re-throttles after
~3.4us of PE idle, halving the PE clock, so stalls cost double):
- alibi+mask fully on PE/ACT: scores arrive in 1/sigma units (Wq
  pre-scaled on host); the per-key term sigma*k rides the ACT exp's
  per-partition bias AP with scale=sigma (both fp32-exact); the
  per-query term + causal mask enter as a second accumulating matmul
  over both halves at once (threshold matrix L builds -MBIG*relu(k-q)
  ramps on the diagonal half + a rank-1 alibi row on both halves).
- per (head, qcol): scores for prior+diag key chunks land in one
  [128,256] PSUM tile, one mask matmul, ONE exp, one DVE add to form
  the denominator operand, two accumulating PV matmuls.
- softmax denom: all-ones matmul (den replicated on all 128
  partitions) -> DVE reciprocal_approx_fast -> DVE multiply. ACT only
  ever runs Exp/Copy: one table load total.
- software pipelining: engines execute their queues IN ORDER, so the
  output projection of block t is interleaved per-(head,qcol) into
  block t+1's attention; the 4 Wo matmuls of one output chunk fill the
  PE while ACT runs exp, and no engine ever waits on the softmax tail.
- the wq weight DMA is split per 128-contraction chunk and interleaved
  with block 0's x strip so the first Q matmul starts as soon as chunk
  0 lands instead of waiting for the full 2MB tile.
"""

import math
from contextlib import ExitStack

import ml_dtypes
import numpy as np

import concourse.bass as bass
import concourse.mybir as mybir
import concourse.tile as tile
from concourse import bacc
from concourse.bass_utils import run_bass_kernel_spmd

B, T, C = 4, 2048, 2048
HD = 128          # head dim
HPC = 4           # Q heads per core
QB = 512          # query block (projection tile free dim)
KC = 128          # key chunk / query column
NQB = T // QB     # 4
NCC = C // 128    # 16 contraction chunks for projections
MBIG = 128.0      # mask ramp step in 1/sigma logit units (bf16-exact)

F32 = mybir.dt.float32
BF16 = mybir.dt.bfloat16
EXP = mybir.ActivationFunctionType.Exp

_CACHE = {}


def build_kernel():
    nc = bacc.Bacc(
        "TRN2",
        target_bir_lowering=False,
        debug=False,
        enable_asserts=False,
        num_devices=8,
    )
    xT2 = nc.dram_tensor("xT2", [2, C, T], BF16, kind="ExternalInput").ap()
    wq_d = nc.dram_tensor("wq", [C, HPC * HD], BF16, kind="ExternalInput").ap()
    wk_d = nc.dram_tensor("wk", [C, HD], BF16, kind="ExternalInput").ap()
    wv_d = nc.dram_tensor("wv", [C, HD], BF16, kind="ExternalInput").ap()
    wo_d = nc.dram_tensor("wo", [HPC * HD, C], BF16, kind="ExternalInput").ap()
    L_d = nc.dram_tensor("Lmat", [KC, KC], BF16, kind="ExternalInput").ap()
    U_d = nc.dram_tensor("Umat", [KC, 2 * KC], BF16, kind="ExternalInput").ap()
    bias_d = nc.dram_tensor("biask", [KC, 1], F32, kind="ExternalInput").ap()
    sig_d = nc.dram_tensor("sigmas", [KC, 1], F32, kind="ExternalInput").ap()
    id_d = nc.dram_tensor("ident", [128, 128], BF16, kind="ExternalInput").ap()
    on_d = nc.dram_tensor("onesc", [128, 128], BF16, kind="ExternalInput").ap()
    outT = nc.dram_tensor("outT", [2, C, T], BF16, kind="ExternalOutput").ap()

    wq_r = wq_d.rearrange("(cc p) d -> p cc d", p=128)

    with ExitStack() as ctx:
        tc = ctx.enter_context(tile.TileContext(nc))
        ctx.enter_context(
            nc.allow_low_precision(reason="bf16 inputs, fp32 accumulate")
        )

        consts = ctx.enter_context(tc.tile_pool(name="consts", bufs=1))
        xpool = ctx.enter_context(tc.tile_pool(name="xpool", bufs=34))
        kvpool = ctx.enter_context(tc.tile_pool(name="kvpool", bufs=1))
        qpool = ctx.enter_context(tc.tile_pool(name="qpool", bufs=3))
        ypool = ctx.enter_context(tc.tile_pool(name="ypool", bufs=2))
        apool = ctx.enter_context(tc.tile_pool(name="apool", bufs=4))
        stpool = ctx.enter_context(tc.tile_pool(name="stpool", bufs=4))
        ppool = ctx.enter_context(tc.tile_pool(name="ppool", bufs=6))
        opool = ctx.enter_context(tc.tile_pool(name="opool", bufs=6))
        rpool = ctx.enter_context(tc.tile_pool(name="rpool", bufs=3))

        ps_acc = ctx.enter_context(tc.tile_pool(name="ps_acc", bufs=2, space="PSUM"))
        ps_s = ctx.enter_context(tc.tile_pool(name="ps_s", bufs=2, space="PSUM"))
        ps_y = ctx.enter_context(tc.tile_pool(name="ps_y", bufs=2, space="PSUM"))
        ps_d = ctx.enter_context(tc.tile_pool(name="ps_d", bufs=2, space="PSUM"))

        # wq arrives per 128-chunk, interleaved with block 0's x strip:
        # the first Q matmul gates only on chunk 0 of each, so the PE
        # starts ~1-2 chunk-DMAs in instead of after the full 2MB wq.
        wq_sb = consts.tile([128, NCC, HPC * HD], BF16)
        xts0 = []
        for cc in range(NCC):
            nc.sync.dma_start(wq_sb[:, cc, :], wq_r[:, cc, :])
            xt = xpool.tile([128, QB], BF16, tag="x")
            nc.sync.dma_start(xt, xT2[0, cc * 128:(cc + 1) * 128, 0:QB])
            xts0.append(xt)

        wk_sb = consts.tile([128, NCC, HD], BF16)
        nc.sync.dma_start(wk_sb, wk_d.rearrange("(cc p) d -> p cc d", p=128))
        wv_sb = consts.tile([128, NCC, HD], BF16)
        nc.sync.dma_start(wv_sb, wv_d.rearrange("(cc p) d -> p cc d", p=128))
        bias_sb = consts.tile([128, 1], F32)
        nc.sync.dma_start(bias_sb, bias_d)
        sig_sb = consts.tile([128, 1], F32)
        nc.sync.dma_start(sig_sb, sig_d)
        ident = consts.tile([128, 128], BF16)
        nc.sync.dma_start(ident, id_d)
        ones = consts.tile([128, 128], BF16)
        nc.sync.dma_start(ones, on_d)
        L_sb = consts.tile([128, KC], BF16)
        nc.sync.dma_start(L_sb, L_d)
        U_sb = consts.tile([128, 2 * KC], BF16)
        nc.sync.dma_start(U_sb, U_d)
        wo_sb = consts.tile([128, HPC, C], BF16)
        nc.sync.dma_start(wo_sb, wo_d.rearrange("(hc p) c -> p hc c", p=128))

        # K^T / V ring buffers: attention only needs a 5-chunk causal window
        # (1 prior + 4 in-block); 6 slots give one chunk of WAR slack.
        RING = 6
        kt_ring = kvpool.tile([128, RING, KC], BF16, tag="kt")
        v_ring = kvpool.tile([128, RING, HD], BF16, tag="v")

        def emit_oproj_group(prev, co):
            """One output chunk (4 Wo matmuls + evac + store) of the
            deferred output projection for `prev` = (b, t0, y_sb)."""
            pb, pt0, py = prev
            o_ps = ps_acc.tile([128, QB], F32, tag="acc")
            for hc in range(HPC):
                nc.tensor.matmul(
                    o_ps,
                    lhsT=wo_sb[:, hc, co * 128:(co + 1) * 128],
                    rhs=py[:, hc, :],
                    start=(hc == 0),
                    stop=(hc == HPC - 1),
                )
            o_sb = opool.tile([128, QB], BF16, tag="o")
            if co % 2 == 0:
                nc.scalar.copy(o_sb, o_ps)
            else:
                nc.vector.tensor_copy(o_sb, o_ps)
            nc.sync.dma_start(
                outT[pb, co * 128:(co + 1) * 128, pt0:pt0 + QB], o_sb
            )

        prev = None  # (b, t0, y_sb) awaiting output projection
        for b in range(2):
            for tb in range(NQB):
                t0 = tb * QB
                # ---- load x^T strip for this block ----
                if b == 0 and tb == 0:
                    xts = xts0
                else:
                    xts = []
                    for cc in range(NCC):
                        xt = xpool.tile([128, QB], BF16, tag="x")
                        nc.sync.dma_start(
                            xt, xT2[b, cc * 128:(cc + 1) * 128, t0:t0 + QB]
                        )
                        xts.append(xt)

                # ---- projections for this block ----
                qT_sb = qpool.tile([128, HPC, QB], BF16)
                for h in range(HPC):
                    ps = ps_acc.tile([128, QB], F32, tag="acc")
                    for cc in range(NCC):
                        nc.tensor.matmul(
                            ps,
                            lhsT=wq_sb[:, cc, h * HD:(h + 1) * HD],
                            rhs=xts[cc],
                            start=(cc == 0),
                            stop=(cc == NCC - 1),
                        )
                    nc.vector.tensor_copy(qT_sb[:, h, :], ps)

                ps = ps_acc.tile([128, QB], F32, tag="acc")
                for cc in range(NCC):
                    nc.tensor.matmul(
                        ps, lhsT=wk_sb[:, cc, :], rhs=xts[cc],
                        start=(cc == 0), stop=(cc == NCC - 1),
                    )
                for kc in range(4):
                    nc.scalar.copy(
                        kt_ring[:, (tb * 4 + kc) % RING, :],
                        ps[:, kc * KC:(kc + 1) * KC],
                    )

                ps = ps_acc.tile([128, QB], F32, tag="acc")
                for cc in range(NCC):
                    nc.tensor.matmul(
                        ps, lhsT=wv_sb[:, cc, :], rhs=xts[cc],
                        start=(cc == 0), stop=(cc == NCC - 1),
                    )
                vT_tmp = stpool.tile([128, QB], BF16, tag="vt")
                nc.scalar.copy(vT_tmp, ps)
                for kc in range(4):
                    tp = ps_s.tile([128, KC], BF16, tag="s")
                    nc.tensor.transpose(tp, vT_tmp[:, kc * KC:(kc + 1) * KC], ident)
                    nc.vector.tensor_copy(v_ring[:, (tb * 4 + kc) % RING, :], tp)

                # ---- attention: per (head, 128-query column), key window
                # = {prior chunk, diagonal chunk}; the previous block's
                # output projection is interleaved per-(h,qcol) so the PE
                # stays busy while ACT runs exp ----
                y_sb = ypool.tile([128, HPC, QB], BF16)
                ci = 0
                for h in range(HPC):
                    y_ps = ps_y.tile([128, QB], F32, tag="y")
                    acc = apool.tile([128, QB], BF16, tag="a")
                    for c in range(4):
                        kb = 4 * tb + c          # diagonal key chunk
                        has_prior = kb > 0
                        qsl = qT_sb[:, h, c * KC:(c + 1) * KC]
                        s_ps = ps_s.tile([128, QB], F32, tag="s")
                        lo = 0 if has_prior else KC
                        # alibi (rank-1 per-q row) + causal mask ramp over
                        # both halves at once. Emitted FIRST with
                        # start=True: a start marks the whole 2KB PSUM bank
                        # pending-zero, so it must precede every other
                        # write to this bank this round.
                        nc.tensor.matmul(
                            s_ps[:, lo:2 * KC], lhsT=L_sb, rhs=U_sb[:, lo:],
                            start=True, stop=False,
                        )
                        if has_prior:
                            nc.tensor.matmul(
                                s_ps[:, 0:KC],
                                lhsT=kt_ring[:, (kb - 1) % RING, :],
                                rhs=qsl, start=False, stop=False,
                            )
                        nc.tensor.matmul(
                            s_ps[:, KC:2 * KC],
                            lhsT=kt_ring[:, kb % RING, :],
                            rhs=qsl, start=False, stop=True,
                        )
                        # fill PE while ACT computes exp of this column
                        if prev is not None and ci < 16:
                            emit_oproj_group(prev, ci)
                        ci += 1
                        pT = ppool.tile([128, 2 * KC], BF16, tag="p")
                        nc.scalar.activation(
                            pT[:, lo:], s_ps[:, lo:2 * KC], EXP,
                            bias=bias_sb[:, 0:1], scale=sig_sb[:, 0:1],
                        )
                        asl = acc[:, c * KC:(c + 1) * KC]
                        if has_prior:
                            nc.vector.tensor_add(asl, pT[:, 0:KC], pT[:, KC:])
                        else:
                            nc.vector.tensor_copy(asl, pT[:, KC:])
                        # y_ps bank: single start (first PV of the head)
                        # marks the bank; later qcols land on pending-zero
                        ysl = y_ps[:, c * KC:(c + 1) * KC]
                        if has_prior:
                            nc.tensor.matmul(
                                ysl, lhsT=v_ring[:, (kb - 1) % RING, :],
                                rhs=pT[:, 0:KC], start=(c == 0), stop=False,
                            )
                        nc.tensor.matmul(
                            ysl, lhsT=v_ring[:, kb % RING, :],
                            rhs=pT[:, KC:],
                            start=(c == 0 and not has_prior), stop=(c == 3),
                        )
                    # normalization: den replicated over partitions via
                    # all-ones matmul, fast Newton reciprocal on DVE
                    # (no ACT, no table switches), multiply on DVE.
                    den_ps = ps_d.tile([128, QB], F32, tag="d")
                    nc.tensor.matmul(den_ps, lhsT=ones, rhs=acc)
                    rec = rpool.tile([128, QB], F32, tag="rec")
                    nc.vector.reciprocal_approx_fast(rec, den_ps)
                    nc.vector.tensor_mul(y_sb[:, h, :], y_ps, rec)

                prev = (b, t0, y_sb)

        # drain: output projection of the final block
        for co in range(16):
            emit_oproj_group(prev, co)

    nc.compile()
    return nc


def make_mask_consts():
    """L [128,128] and U [128,256] bf16 constants for the mask/alibi
    matmul, in 1/sigma logit units (scores arrive pre-scaled by 1/sigma
    via Wq; the ACT exp applies scale=sigma in fp32). Columns 0:128 are
    the prior-chunk half, 128:256 the diagonal half; q is the local
    query index within the column:
    (L.T @ U)[k, q_prior] = -128 - q          [alibi row only]
    (L.T @ U)[k, q_diag]  = -MBIG*max(0, k-q) - q
    Row j<127 of L/U is the threshold pair [k >= j+1]*[j >= q]; row 127
    carries the rank-1 alibi term. All values are small integers
    (bf16-exact)."""
    BF = ml_dtypes.bfloat16
    j = np.arange(KC)[:, None]
    k = np.arange(KC)[None, :]
    q = np.arange(KC)[None, :]
    L = np.zeros((KC, KC), np.float32)
    L[:127] = -MBIG * (k >= (j[:127] + 1))
    L[127, :] = 1.0
    U = np.zeros((KC, 2 * KC), np.float32)
    U[127, 0:KC] = -128.0 - q
    U[:127, KC:] = (j[:127] >= q)
    U[127, KC:] = -q
    return L.astype(BF), U.astype(BF)


def kernel(x, Wq, Wk, Wv, Wo):
    import os
    import time

    dbg = os.environ.get("KERNEL_DEBUG") == "1"
    t0 = time.time()

    def tick(msg):
        nonlocal t0
        if dbg:
            print(f"[kernel] {msg}: {time.time() - t0:.2f}s", flush=True)
        t0 = time.time()

    x = np.ascontiguousarray(np.asarray(x, np.float32))
    Wq = np.ascontiguousarray(np.asarray(Wq, np.float32))
    Wk = np.ascontiguousarray(np.asarray(Wk, np.float32))
    Wv = np.ascontiguousarray(np.asarray(Wv, np.float32))
    Wo = np.ascontiguousarray(np.asarray(Wo, np.float32))

    tick("input prep")
    if "nc" not in _CACHE:
        _CACHE["nc"] = build_kernel()
        tick("build_kernel")
    nc = _CACHE["nc"]

    s = 1.0 / math.sqrt(HD)
    slopes = [2.0 ** -0.5, 0.5, 2.0 ** -1.5, 0.25]
    BF = ml_dtypes.bfloat16
    ident = np.eye(128, dtype=BF)

    L, U = make_mask_consts()
    in_maps = []
    for c in range(8):
        bg, g = c // 4, c % 4
        xT2 = np.stack(
            [np.ascontiguousarray(x[2 * bg + i].T) for i in range(2)]
        )
        biask = (slopes[g] * np.arange(KC, dtype=np.float32))[:, None]
        in_maps.append({
            "xT2": xT2.astype(BF),
            "wq": (Wq[:, g * 512:(g + 1) * 512] * (s / slopes[g])).astype(BF),
            "wk": Wk[:, g * HD:(g + 1) * HD].astype(BF),
            "wv": Wv[:, g * HD:(g + 1) * HD].astype(BF),
            "wo": Wo[g * 512:(g + 1) * 512, :].astype(BF),
            "Lmat": L,
            "Umat": U,
            "biask": biask,
            "sigmas": np.full((KC, 1), slopes[g], np.float32),
            "ident": ident,
            "onesc": np.ones((128, 128), BF),
        })

    tick("in_maps prep")
    res = run_bass_kernel_spmd(nc, in_maps, core_ids=list(range(8)))
    tick("device run")
    out = np.zeros((B, T, C), np.float32)
    for c in range(8):
        bg, g = c // 4, c % 4
        oT = np.asarray(res.results[c]["outT"], np.float32)
        for i in range(2):
            out[2 * bg + i] += oT[i].T
    tick("gather")
    return out
